# revision 17
# baseline (speedup 1.0000x reference)
"""Multi-head attention (SEQ=4096, d_model=1024, 16 heads of d=64) on 8 TRN2
NeuronCores, tensor-parallel over heads (2 heads/core), AllToAll re-shard to
sequence-parallel before the output projection.

v2: the whole attention phase runs in the PE's 64x128 row-tiled mode — every
matmul slot is a concurrent (T0, T8) tile pair, so the K=64-per-head scores
and the K=128 AV contraction both stream at full column rate:
  scores: T0 computes head-h scoresT for key block 2bp (contraction = the 64
    head dims, array rows 0-63), T8 computes block 2bp+1 on rows 64-127 using
    partition-swapped copies of qhT/khT built during the projection drains.
  AV: T0 accumulates keys 0-63 of a block into avx[65,512], T8 keys 64-127
    into avy (separate PSUM banks; halves summed by one DVE add at qc end).
  3 pair-slots per block pair vs 4 serial matmuls before -> 25% less PE time.
exp split: ACT takes pair-member a, DVE member b (Schraudolph bf16 bit-trick).
Softmax denominators ride the AV ones-column; per-head reciprocals
(reciprocal_approx_fast on the packed [8,512] sums) are shipped as a 65th
bf16 row of each AllToAll shard -- one collective per head instead of two,
nothing serialized behind the 512KB dv AllToAll.
FC: head-0 scale + 6 of 8 passA tiles fill the exposed head-1 AllToAll
window; residual tiles are fetched mid-attention; relu on ACT, +residual on
DVE, per-tile output DMA.
"""

import os
import sys

sys.path.insert(0, "/opt/trn_rl_repo")

import numpy as np
import ml_dtypes

import concourse.bass as bass
import concourse.mybir as mybir
import concourse.tile as tile
from concourse import bacc
from concourse.bass_utils import run_bass_kernel_spmd

SEQ = 4096
DM = 1024
NH = 16
DK = 64
DV = 64
CORES = 8
P = 128
HL = 2 * DK  # 128: two heads' head-dim per core
SROWS = SEQ // CORES  # 512 output rows per core
MO = DM // P  # 8 m-chunks of d_model
F32 = mybir.dt.float32
BF16 = mybir.dt.bfloat16

EXP_RB = int(os.environ.get("EXP_RB", "0"))
AV_LEAD = int(os.environ.get("AV_LEAD", "2"))  # every RB-th pair member b -> ACT
EXP_A = 128.0 / float(np.log(2.0))  # bf16-bits Schraudolph slope
EXP_B = 16256.0 - 5.5  # 127*128 - C


def _exp_tile(nc, out_bf16, in_psum, scale, use_dve):
    """out = exp(scale * in), bf16."""
    if use_dve:
        nc.vector.tensor_scalar(
            out=out_bf16.bitcast(mybir.dt.int16),
            in0=in_psum,
            scalar1=float(scale * EXP_A),
            scalar2=float(EXP_B),
            op0=mybir.AluOpType.mult,
            op1=mybir.AluOpType.add,
        )
    else:
        nc.scalar.activation(
            out=out_bf16,
            in_=in_psum,
            func=mybir.ActivationFunctionType.Exp,
            scale=float(scale),
        )


def build(seq=SEQ):
    srows = seq // CORES
    kb = seq // P  # key blocks
    qcw = min(512, seq)
    qcs = seq // qcw
    sb_blocks = srows // P

    nc = bacc.Bacc(
        "TRN2",
        target_bir_lowering=False,
        debug=False,
        enable_asserts=True,
        num_devices=CORES,
    )

    qT = nc.dram_tensor("qT", [DM, seq], BF16, kind="ExternalInput").ap()
    kT = nc.dram_tensor("kT", [DM, seq], BF16, kind="ExternalInput").ap()
    vT = nc.dram_tensor("vT", [DM, seq], BF16, kind="ExternalInput").ap()
    wqT = nc.dram_tensor("wqT", [DM, HL], BF16, kind="ExternalInput").ap()
    wkT = nc.dram_tensor("wkT", [DM, HL], BF16, kind="ExternalInput").ap()
    wvT = nc.dram_tensor("wvT", [DM, HL], BF16, kind="ExternalInput").ap()
    # pre-permuted on host to match the post-A2A dv row order
    wfcT = nc.dram_tensor("wfcT", [DM, DM], BF16, kind="ExternalInput").ap()
    qres = nc.dram_tensor("qres", [srows, DM], F32, kind="ExternalInput").ap()
    sel_in = nc.dram_tensor("sel", [2, P], BF16, kind="ExternalInput").ap()
    out = nc.dram_tensor("out", [srows, DM], F32, kind="ExternalOutput").ap()

    qT_r = qT.rearrange("(o p) s -> p o s", p=P)
    kT_r = kT.rearrange("(o p) s -> p o s", p=P)
    vT_r = vT.rearrange("(o p) s -> p o s", p=P)
    wqT_r = wqT.rearrange("(o p) h -> p o h", p=P)
    wkT_r = wkT.rearrange("(o p) h -> p o h", p=P)
    wvT_r = wvT.rearrange("(o p) h -> p o h", p=P)
    wfcT_r = wfcT.rearrange("(o p) d -> p o d", p=P)
    qres_r = qres.rearrange("(b p) d -> p b d", p=P)
    out_r = out.rearrange("(b p) d -> p b d", p=P)

    with tile.TileContext(nc) as tc:
        with (
            tc.tile_pool(name="const", bufs=1) as cpool,
            tc.tile_pool(name="xin", bufs=8) as xpool,
            tc.tile_pool(name="pt", bufs=8) as ptpool,
            tc.tile_pool(name="small", bufs=3) as spool,
            tc.tile_pool(name="smp", bufs=1) as smpool,
            tc.tile_pool(name="ps", bufs=8, space="PSUM") as ps,
            tc.tile_pool(name="dram", bufs=1, space="DRAM") as dr,
        ):
            # ---- constants / persistent tiles ----
            wq_sb = cpool.tile([P, MO, HL], BF16, tag="wq")
            wk_sb = cpool.tile([P, MO, HL], BF16, tag="wk")
            wv_sb = cpool.tile([P, MO, HL], BF16, tag="wv")
            nc.scalar.dma_start(wq_sb[:], wqT_r[:])
            nc.scalar.dma_start(wk_sb[:], wkT_r[:])
            nc.scalar.dma_start(wv_sb[:], wvT_r[:])

            # natural (rows 0:64 = head0) and partition-swapped copies
            qh2 = cpool.tile([P, seq], BF16, tag="qh2")
            qhs = cpool.tile([P, seq], BF16, tag="qhs")
            kh2 = cpool.tile([P, seq], BF16, tag="kh2")
            khs = cpool.tile([P, seq], BF16, tag="khs")
            vh = [
                cpool.tile([P, kb, DV + 1], BF16, tag=f"vh{h}", name=f"vh{h}")
                for h in range(2)
            ]
            nc.vector.memset(vh[0][:, :, DV : DV + 1], 1.0)
            nc.vector.memset(vh[1][:, :, DV : DV + 1], 1.0)
            outT = cpool.tile([P, seq], BF16, tag="outT")
            sel = cpool.tile([2, P], BF16, tag="sel")

            # ---- phase 1: projections ----
            def load_chunks(src_r):
                xts = []
                engs = [nc.sync, nc.gpsimd, nc.scalar]
                for o in range(MO):
                    xt = xpool.tile([P, seq], BF16, tag="xin", name=f"xin{o}")
                    engs[o % 3].dma_start(xt[:], src_r[:, o, :])
                    xts.append(xt)
                return xts

            pgroups = seq // 512 if seq >= 512 else 1
            pgw = seq // pgroups
            for w_sb, src_r, nat, swp in (
                (wq_sb, qT_r, qh2, qhs),
                (wk_sb, kT_r, kh2, khs),
            ):
                xts = load_chunks(src_r)
                pps = [
                    ps.tile([P, pgw], F32, tag="ps", name=f"pp{g}")
                    for g in range(pgroups)
                ]
                for o in range(MO):
                    for g in range(pgroups):
                        nc.tensor.matmul(
                            pps[g][:HL, :],
                            w_sb[:, o, :],
                            xts[o][:, g * pgw :][:, :pgw],
                            start=(o == 0),
                            stop=(o == MO - 1),
                        )
                for g in range(pgroups):
                    gsl = slice(g * pgw, (g + 1) * pgw)
                    nc.scalar.copy(out=nat[:, gsl], in_=pps[g][:HL])
                    # swapped copy: head1 -> rows 0:64, head0 -> rows 64:128
                    nc.vector.tensor_copy(out=swp[0:DK, gsl], in_=pps[g][DK:HL])
                    nc.vector.tensor_copy(out=swp[DK:HL, gsl], in_=pps[g][0:DK])
            xts = load_chunks(vT_r)
            # o-major so each chunk's 32 block-MMs run as the chunk lands
            # (b-major would serialize ~27us of matmuls after the last chunk
            # in the in-order Tensor queue). 4 accumulators share a PSUM slot.
            pvq = [
                ps.tile([P, 4, HL], F32, tag="ps", name=f"pvq{g}")
                for g in range(kb // 4)
            ]
            for o in range(MO):
                for b in range(kb):
                    # start=True clears has_written for the whole PSUM bank:
                    # only slice 0 may issue it, the other slices' first
                    # writes overwrite their (cleared, unwritten) regions
                    nc.tensor.matmul(
                        pvq[b // 4][:, b % 4, :],
                        xts[o][:, b * P : (b + 1) * P],
                        wv_sb[:, o, :],
                        start=(o == 0 and b % 4 == 0),
                        stop=(o == MO - 1),
                        skip_group_check=(b % 4 != 0),
                    )
            for g in range(kb // 4):
                eng = nc.scalar if g % 2 == 0 else nc.vector
                gs = slice(4 * g, 4 * g + 4)
                if g % 2 == 0:
                    nc.scalar.copy(out=vh[0][:, gs, :DV], in_=pvq[g][:, :, :DK])
                    nc.vector.tensor_copy(out=vh[1][:, gs, :DV],
                                          in_=pvq[g][:, :, DK:HL])
                else:
                    nc.vector.tensor_copy(out=vh[0][:, gs, :DV],
                                          in_=pvq[g][:, :, :DK])
                    nc.scalar.copy(out=vh[1][:, gs, :DV], in_=pvq[g][:, :, DK:HL])

            # late constants: after projection inputs so they don't delay them
            nc.sync.dma_start(sel[:], sel_in[:])
            wfc_sb = cpool.tile([P, MO, DM], BF16, tag="wfc")
            nc.sync.dma_start(wfc_sb[:], wfcT_r[:])
            qres_sb = {}
            for sb in range(sb_blocks):
                qre = spool.tile([P, DM], F32, tag="qre", bufs=sb_blocks,
                                 name=f"qre{sb}")
                nc.sync.dma_start(qre[:], qres_r[:, sb, :])
                qres_sb[sb] = qre

            # ---- phase 2+3: attention (all 64x128-mode pair slots) ----
            a2a_in, a2a_out = [], []
            hchunks = (CORES * DK) // P  # 4 fc lhsT chunks per head
            ofull, recips = [None, None], [None, None]

            def _fc_load(h):
                of = cpool.tile([P, hchunks, srows], BF16, tag=f"of{h}",
                                name=f"of{h}")
                rc = smpool.tile([2, hchunks, srows], BF16, tag=f"rc{h}",
                                 name=f"rc{h}")
                dvo = a2a_out[h]
                for o in range(hchunks):
                    for g in range(2):
                        j = 2 * o + g
                        nc.sync.dma_start(
                            of[DK * g : DK * (g + 1), o, :],
                            dvo[(DV + 1) * j : (DV + 1) * j + DV, :],
                        )
                        nc.sync.dma_start(
                            rc[g : g + 1, o, :],
                            dvo[(DV + 1) * j + DV : (DV + 1) * (j + 1), :],
                        )
                ofull[h] = of
                recips[h] = rc

            def _fc_scale(h, sel_t=None):
                # bc broadcasts the bf16 SUMS; invert with the fast custom-DVE
                # reciprocal (128 lanes) and multiply in place
                sel_t = sel if sel_t is None else sel_t
                for o in range(hchunks):
                    bc = ps.tile([P, srows], F32, tag="ps", name=f"bc{h}{o}")
                    nc.tensor.matmul(
                        bc[:], sel_t[:], recips[h][:, o, :], start=True, stop=True
                    )
                    rsc = spool.tile([P, srows], F32, tag="rsc", bufs=2,
                                     name=f"rsc{h}{o}")
                    nc.vector.reciprocal_approx_fast(out=rsc[:], in_=bc[:])
                    nc.vector.tensor_mul(
                        out=ofull[h][:, o, :], in0=ofull[h][:, o, :], in1=rsc[:]
                    )

            for h in range(2):
                hs = h * DK
                klo, qlo = (kh2, qh2) if h == 0 else (khs, qhs)
                khi, qhi = (khs, qhs) if h == 0 else (kh2, qh2)
                sums_q = {}
                pend_drain = None
                for qc in range(qcs):
                    q0 = qc * qcw
                    avx = ps.tile([DV + 1, qcw], F32, tag="ps",
                                  name=f"avx{h}_{qc}")
                    avy = ps.tile([DV + 1, qcw], F32, tag="ps",
                                  name=f"avy{h}_{qc}")
                    # software pipeline: the Tensor queue is in-order, so AV(bp)
                    # sitting right after sco(bp) would stall the PE on that
                    # bp's exp. Emit scores LEAD block-pairs ahead of the AVs.
                    pts = {}
                    nbp = kb // 2
                    for step in range(nbp + AV_LEAD):
                        if step == 1 and pend_drain is not None:
                            pend_drain()
                            pend_drain = None
                        if step < nbp:
                            bp = step
                            b0, b1 = 2 * bp, 2 * bp + 1
                            sca = ps.tile([P, qcw], F32, tag="ps",
                                          name=f"sca{h}_{qc}_{bp}")
                            scb = ps.tile([P, qcw], F32, tag="ps",
                                          name=f"scb{h}_{qc}_{bp}")
                            nc.tensor.matmul(
                                sca[:],
                                klo[0:DK, b0 * P : (b0 + 1) * P],
                                qlo[0:DK, q0 : q0 + qcw],
                                start=True,
                                stop=True,
                                tile_position=(0, 0),
                            )
                            nc.tensor.matmul(
                                scb[:],
                                khi[DK:HL, b1 * P : (b1 + 1) * P],
                                qhi[DK:HL, q0 : q0 + qcw],
                                start=True,
                                stop=True,
                                tile_position=(64, 0),
                            )
                            pta = ptpool.tile([P, qcw], BF16, tag="pt",
                                              name=f"pta{h}_{qc}_{bp}")
                            ptb = ptpool.tile([P, qcw], BF16, tag="pt",
                                              name=f"ptb{h}_{qc}_{bp}")
                            _exp_tile(nc, pta[:], sca[:], 1.0 / np.sqrt(DK),
                                      False)
                            b_on_act = EXP_RB > 0 and (bp % EXP_RB) == EXP_RB - 1
                            _exp_tile(nc, ptb[:], scb[:], 1.0 / np.sqrt(DK),
                                      not b_on_act)
                            pts[bp] = (pta, ptb)
                        if step >= AV_LEAD:
                            bp2 = step - AV_LEAD
                            pta, ptb = pts.pop(bp2)
                            for blk, pt in ((2 * bp2, pta), (2 * bp2 + 1, ptb)):
                                nc.tensor.matmul(
                                    avx[:],
                                    vh[h][0:DK, blk, :],
                                    pt[0:DK, :],
                                    start=(blk == 0),
                                    stop=(blk == kb - 1),
                                    tile_position=(0, 0),
                                )
                                nc.tensor.matmul(
                                    avy[:],
                                    vh[h][DK:HL, blk, :],
                                    pt[DK:HL, :],
                                    start=(blk == 0),
                                    stop=(blk == kb - 1),
                                    tile_position=(64, 0),
                                )
                    # drain closure: DVE cannot read two PSUM inputs in one
                    # op, so ACT stages avx in SBUF and DVE adds avy on top.
                    # Deferred one step into the next qc so the refill exps
                    # aren't stuck behind the drains in the ACT/DVE FIFOs.
                    def _drain(avx=avx, avy=avy, h=h, qc=qc, q0=q0):
                        stg = spool.tile([DV + 1, qcw], F32, tag="stg", bufs=2,
                                         name=f"stg{h}_{qc}")
                        nc.scalar.copy(out=stg[:], in_=avx[:])
                        nc.vector.tensor_add(
                            out=outT[h * DK : h * DK + DK, q0 : q0 + qcw],
                            in0=stg[0:DV, :],
                            in1=avy[0:DV, :],
                        )
                        # bf16 sums row per qc: shard gathers stream during
                        # attention (no per-head recip serializer)
                        sq = spool.tile([1, qcw], BF16, tag="sq", bufs=qcs,
                                        name=f"sq{h}_{qc}")
                        sums_q[qc] = sq
                        nc.vector.tensor_add(
                            out=sq[:],
                            in0=stg[DV : DV + 1, :],
                            in1=avy[DV : DV + 1, :],
                        )
                    if qc == qcs - 1:
                        _drain()
                    else:
                        pend_drain = _drain
                    if h == 1 and qc == qcs - 3:
                        _fc_load(0)
                dvi = dr.tile([CORES * (DV + 1), srows], BF16, name=f"a2ai{h}")
                dvo = dr.tile([CORES * (DV + 1), srows], BF16, name=f"a2ao{h}")
                for j in range(CORES):
                    nc.sync.dma_start(
                        dvi[(DV + 1) * j : (DV + 1) * j + DV, :],
                        outT[hs : hs + DK, j * srows : (j + 1) * srows],
                    )
                    nc.sync.dma_start(
                        dvi[(DV + 1) * j + DV : (DV + 1) * (j + 1), :],
                        sums_q[j][:],
                    )
                nc.gpsimd.collective_compute(
                    "AllToAll",
                    mybir.AluOpType.bypass,
                    replica_groups=[list(range(CORES))],
                    ins=[dvi.opt()],
                    outs=[dvo.opt()],
                )
                a2a_in.append(dvi)
                a2a_out.append(dvo)

            # ---- phase 4: FC + epilogue ----
            # Gate the head-0 FC chain on a token DMA'd from the LAST shard of
            # head-1's A2A input: it lands right when the collective triggers,
            # so the scheduler cannot hoist scale0/passA into attention — they
            # run inside the exposed A2A window instead.
            tok = spool.tile([1, 4], BF16, tag="tok", bufs=1)
            nc.sync.dma_start(
                tok[:], a2a_in[1][(DV + 1) * (CORES - 1) : (DV + 1) * (CORES - 1) + 1, 0:4]
            )
            sel2 = spool.tile([2, P], BF16, tag="sel2", bufs=1)
            nc.vector.tensor_copy(out=sel2[:], in_=sel[:])
            # sel[0, 0:4] is 1.0; rewrite it as tok*0 + 1 to carry the dep
            nc.vector.tensor_scalar(
                out=sel2[0:1, 0:4],
                in0=tok[:],
                scalar1=0.0,
                scalar2=1.0,
                op0=mybir.AluOpType.mult,
                op1=mybir.AluOpType.add,
            )
            # head-0 scale + first 6 passA tiles fill the head-1 A2A window
            tiles_fc = [(sb, nm) for sb in range(sb_blocks) for nm in range(DM // 512)]
            groups = [tiles_fc[0:4], tiles_fc[4:7], tiles_fc[7:8]]
            pfs_all = {}

            def _fc_passA(sb, nm):
                pf = ps.tile([P, 512], F32, tag="ps", name=f"pf{sb}_{nm}")
                pfs_all[(sb, nm)] = pf
                for o in range(hchunks):
                    nc.tensor.matmul(
                        pf[:],
                        ofull[0][:, o, sb * P : (sb + 1) * P],
                        wfc_sb[:, o, nm * 512 : (nm + 1) * 512],
                        start=(o == 0),
                        stop=False,
                    )

            def _fc_passB_epi(sb, nm):
                pf = pfs_all[(sb, nm)]
                for o in range(hchunks):
                    nc.tensor.matmul(
                        pf[:],
                        ofull[1][:, o, sb * P : (sb + 1) * P],
                        wfc_sb[:, hchunks + o, nm * 512 : (nm + 1) * 512],
                        start=False,
                        stop=(o == hchunks - 1),
                    )
                eo = spool.tile([P, 512], F32, tag="eo")
                nc.scalar.activation(
                    out=eo[:], in_=pf[:], func=mybir.ActivationFunctionType.Relu
                )
                nc.vector.tensor_add(
                    out=eo[:],
                    in0=eo[:],
                    in1=qres_sb[sb][:, nm * 512 : (nm + 1) * 512],
                )
                nc.sync.dma_start(out_r[:, sb, nm * 512 : (nm + 1) * 512], eo[:])

            _fc_scale(0, sel2)
            for gi in (0, 1):
                for sb, nm in groups[gi]:
                    _fc_passA(sb, nm)
            _fc_load(1)
            _fc_scale(1)
            # tile 8's head-0 pass takes the bank scale1's broadcasts used
            for sb, nm in groups[2]:
                _fc_passA(sb, nm)
            for gi in (0, 1, 2):
                for sb, nm in groups[gi]:
                    _fc_passB_epi(sb, nm)

    nc.compile()
    return nc


def _fc_perm():
    """Row permutation of WfcT matching the post-A2A dv order: FC lhsT chunk
    o (of head-h stream) partition p holds global dv row
    128*(2o + p//64) + h*64 + (p%64)."""
    perm = []
    for h in range(2):
        for o in range(4):
            for p in range(P):
                perm.append(128 * (2 * o + p // 64) + h * 64 + (p % 64))
    return np.array(perm)


def make_in_maps(q, k, v, Wq, Wk, Wv, Wfc, seq=SEQ):
    srows = seq // CORES
    bf = ml_dtypes.bfloat16
    qT = np.ascontiguousarray(q.T).astype(bf)
    kT = np.ascontiguousarray(k.T).astype(bf)
    vT = np.ascontiguousarray(v.T).astype(bf)
    wfcT = np.ascontiguousarray(Wfc.T[_fc_perm()]).astype(bf)
    sel = np.zeros((2, P), bf)
    sel[0, :DK] = 1.0
    sel[1, DK:] = 1.0
    in_maps = []
    for c in range(CORES):
        sl = slice(c * HL, (c + 1) * HL)
        in_maps.append(
            {
                "qT": qT,
                "kT": kT,
                "vT": vT,
                "wqT": np.ascontiguousarray(Wq[sl].T).astype(bf),
                "wkT": np.ascontiguousarray(Wk[sl].T).astype(bf),
                "wvT": np.ascontiguousarray(Wv[sl].T).astype(bf),
                "wfcT": wfcT,
                "sel": sel,
                "qres": np.ascontiguousarray(q[c * srows : (c + 1) * srows]).astype(
                    np.float32
                ),
            }
        )
    return in_maps


_NC_CACHE = {}


def kernel(q, k, v, Wq, Wk, Wv, Wfc):
    key = "full"
    if key not in _NC_CACHE:
        _NC_CACHE[key] = build()
    nc = _NC_CACHE[key]
    in_maps = make_in_maps(q, k, v, Wq, Wk, Wv, Wfc)
    trace = bool(int(os.environ.get("KERNEL_TRACE", "0")))
    tc_env = os.environ.get("KERNEL_TRACE_CORES", "")
    kw = {}
    if tc_env:
        kw["trace_cores"] = [int(x) for x in tc_env.split(",")]
    res = run_bass_kernel_spmd(nc, in_maps, list(range(CORES)), trace=trace, **kw)
    if trace:
        kernel.last_exec_time_ns = res.exec_time_ns
        kernel.last_profile = res
    out = np.concatenate([res.results[c]["out"] for c in range(CORES)], axis=0)
    return out.astype(np.float32)
